# revision 11
# baseline (speedup 1.0000x reference)
"""GroupedQueryAttention TRN2 kernel (v4).

Sharding: 4-way tensor-parallel over heads x 2-way data-parallel over batch.
Core c handles batch b=c//4 and head-group gc=c%4 (kv heads {2gc, 2gc+1},
q heads (hkv, g) for g in 0..3 -> 8 q heads per core).

HW reality (microbenched): PE streams ~0.51ns/col for ANY dtype with a
~196ns pipeline-restart cost at dependency waits; ScalarE exp of a
[128,2,512] tile costs ~1.2us and paces phase B. The Tile scheduler
reorders per-engine instructions, so throughput is governed by ring
depths (buffering), not emission order.

Per-core program (T=2048, C=2048, D=64), matmuls bf16 (fp32 PSUM except S):
  A: Q^T/K^T/V^T projections; weight tiles stream from the GpSimd DGE
     queue so the x-tile stream on the Sync queue starts immediately.
  B: S tiles in FP16 PSUM (1 bank each -> ps ring depth 3, deep exp
     lookahead). Causal mask: GpSimd zeroes the exp'd diagonal corners
     (0/1 triangle multiply, latency hidden by the ring). O'^T + rowsum
     accumulated on PE via the V' ones-column. Normalize: DVE rowsum
     copy, PE broadcast into a dedicated 1-bank pru, DVE
     reciprocal_approx_fast, DVE multiply -> oT bf16.
  C: y-groups (4 matmuls into a borrowed ps-ring slot, DVE evac, DMA out)
     interleaved into B to fill ScalarE-bound slack; leftovers drain after.
  Host sums 4 bf16 partials per batch.
"""

import numpy as np
import ml_dtypes

import concourse.bass as bass
import concourse.mybir as mybir
import concourse.tile as tile
from concourse import bacc
from concourse.bass_utils import run_bass_kernel_spmd

H, HKV, D, G = 32, 8, 64, 4
B, T, C = 2, 2048, 2048
P = 128
NCORES = 8
F32 = mybir.dt.float32
FR = mybir.dt.float32r
BF16 = mybir.dt.bfloat16
FP16 = mybir.dt.float16

NT = T // 512   # 4 q blocks of 512
NK = C // P     # 16 contraction tiles
NTT = T // P    # 16 key/row tiles

_NC_CACHE = None


def build_kernel(nc, tc, ins, outs):
    xT, wqT, wkT, wvT, woR = (
        ins["xT"], ins["wqT"], ins["wkT"], ins["wvT"], ins["woR"])
    tri, iden = ins["tri"], ins["iden"]
    y = outs["y"]
    EXP = mybir.ActivationFunctionType.Exp
    CPY = mybir.ActivationFunctionType.Copy

    # ---- persistent SBUF ----
    persist = tc._persist_pool
    qT_sb = persist.tile([P, G, T], BF16, name="qT_sb", tag="qT_sb")
    kT_sb = persist.tile([P, T], BF16, name="kT_sb", tag="kT_sb")
    vTt = persist.tile([P, T], BF16, name="vTt", tag="vTt")
    v_sb = persist.tile([P, NTT, 130], BF16, name="v_sb", tag="v_sb")
    oT_sb = persist.tile([P, G, T], BF16, name="oT_sb", tag="oT_sb")
    wo_sb = persist.tile([P, G, C], BF16, name="wo_sb", tag="wo_sb")
    tri_sb = persist.tile([P, P], BF16, name="tri_sb", tag="tri_sb")
    iden_sb = persist.tile([P, P], BF16, name="iden_sb", tag="iden_sb")
    ones_sb = persist.tile([65, 64], FR, name="ones_sb", tag="ones_sb")

    nc.sync.dma_start(iden_sb[:], iden[:])
    nc.sync.dma_start(tri_sb[:], tri[:])
    nc.sync.dma_start(ones_sb[:], ins["ones64"][:])
    nc.sync.dma_start(v_sb[:], ins["vinit"][:])

    # ================= phase A: projections =================
    with (
        tc.tile_pool(name="wproj", bufs=1) as wpool,
        tc.tile_pool(name="xt", bufs=6) as xpool,
        tc.tile_pool(name="pp", bufs=8, space="PSUM") as pp,
    ):
        wq_sb = wpool.tile([P, NK, 512], BF16, name="wq_sb")
        wk_sb = wpool.tile([P, NK, P], BF16, name="wk_sb")
        wv_sb = wpool.tile([P, NK, P], BF16, name="wv_sb")
        wqr = wqT.rearrange("(ko p) m -> p ko m", p=P)
        wkr = wkT.rearrange("(ko p) m -> p ko m", p=P)
        wvr = wvT.rearrange("(ko p) m -> p ko m", p=P)
        # weight slices stream from the GpSimd DGE queue; x tiles own Sync
        for k in range(NK):
            nc.gpsimd.dma_start(wq_sb[:, k, :], wqr[:, k, :])
            nc.gpsimd.dma_start(wk_sb[:, k, :], wkr[:, k, :])
            nc.gpsimd.dma_start(wv_sb[:, k, :], wvr[:, k, :])

        for tb in range(NT):
            ts = slice(tb * 512, (tb + 1) * 512)
            psq = [pp.tile([P, 512], F32, tag="pp", name=f"psq_{tb}_{g}")
                   for g in range(G)]
            psk = pp.tile([P, 512], F32, tag="pp", name=f"psk_{tb}")
            psv = pp.tile([P, 512], F32, tag="pp", name=f"psv_{tb}")
            for k in range(NK):
                xt = xpool.tile([P, 512], BF16, tag="xt", name=f"xt_{tb}_{k}")
                nc.sync.dma_start(xt[:], xT[k * P:(k + 1) * P, ts])
                st, sp = (k == 0), (k == NK - 1)
                for g in range(G):
                    nc.tensor.matmul(psq[g][:],
                                     wq_sb[:, k, g * P:(g + 1) * P],
                                     xt[:], start=st, stop=sp)
                nc.tensor.matmul(psk[:], wk_sb[:, k, :], xt[:],
                                 start=st, stop=sp)
                nc.tensor.matmul(psv[:], wv_sb[:, k, :], xt[:],
                                 start=st, stop=sp)
            for g in range(G):
                nc.scalar.activation(qT_sb[:, g, ts], psq[g][:], CPY)
            nc.vector.tensor_copy(kT_sb[:, ts], psk[:])
            nc.vector.tensor_copy(vTt[:, ts], psv[:])
            for tt in range(4 * tb, 4 * tb + 4):
                pt_ = pp.tile([P, P], BF16, tag="pp", name=f"pvt_{tt}")
                nc.tensor.transpose(pt_[:], vTt[:, tt * P:(tt + 1) * P],
                                    iden_sb[:])
                nc.vector.tensor_copy(v_sb[:, tt, 0:64], pt_[:, 0:64])
                nc.vector.tensor_copy(v_sb[:, tt, 65:129], pt_[:, 64:128])

    # ============ phases B+C: attention + interleaved output proj ==========
    nc.gpsimd.dma_start(wo_sb[:], woR.rearrange("(m p) c -> p m c", p=P))
    with (
        tc.tile_pool(name="ps", bufs=2, space="PSUM") as pspool,
        tc.tile_pool(name="po", bufs=2, space="PSUM") as popool,
        tc.tile_pool(name="ptp", bufs=8) as ptpool,
        tc.tile_pool(name="rs", bufs=4) as rspool,
        tc.tile_pool(name="rb", bufs=4) as rbpool,
        tc.tile_pool(name="ysb", bufs=8) as ypool,
    ):
        c_queue = [(tt, cb) for tt in range(NTT) for cb in range(NT)]
        c_ready = [0]   # how many c-groups may be emitted (rows normalized)

        def emit_c_group():
            tt, cb = c_queue.pop(0)
            py = pspool.tile([P, 512], F32, tag="ps", name=f"py_{tt}_{cb}")
            for m in range(G):
                nc.tensor.matmul(
                    py[:], oT_sb[:, m, tt * P:(tt + 1) * P],
                    wo_sb[:, m, cb * 512:(cb + 1) * 512],
                    start=(m == 0), stop=(m == G - 1),
                )
            ysb = ypool.tile([P, 512], BF16, tag="ysb", name=f"y_{tt}_{cb}")
            nc.vector.tensor_copy(ysb[:], py[:])
            nc.sync.dma_start(
                y[tt * P:(tt + 1) * P, cb * 512:(cb + 1) * 512], ysb[:])

        for j in range(NT):
            qs0 = j * 512
            n = 4 * j + 4
            for g in range(G):
                po = popool.tile([P, 2, 512], F32, tag="po",
                                 name=f"po_{j}_{g}")
                for i in range(n):
                    loc = max(0, P * i - qs0)
                    diag = i >= 4 * j
                    ps = pspool.tile([P, 2, 512], F32, tag="ps",
                                     name=f"ps_{j}_{g}_{i}")
                    for h in range(2):
                        nc.tensor.matmul(
                            ps[:, h, loc:512],
                            kT_sb[h * 64:(h + 1) * 64, i * P:(i + 1) * P],
                            qT_sb[h * 64:(h + 1) * 64, g, qs0 + loc:qs0 + 512],
                            start=True, stop=not diag,
                        )
                        if diag:  # add -1e5 to the masked corner in PSUM
                            nc.tensor.matmul(
                                ps[:, h, loc:loc + P], iden_sb[:], tri_sb[:],
                                start=False, stop=True,
                            )
                    pt_ = ptpool.tile([P, 2, 512], BF16, tag="pt",
                                      name=f"pt_{j}_{g}_{i}")
                    nc.scalar.activation(pt_[:, :, loc:512],
                                         ps[:, :, loc:512], EXP, scale=0.125)
                    for h in range(2):
                        nc.tensor.matmul(
                            po[0:65, h, loc:512],
                            v_sb[:, i, h * 65:h * 65 + 65],
                            pt_[:, h, loc:512],
                            start=(i == 0), stop=(i == n - 1),
                        )
                    # interleave output-projection groups to fill exp-bound
                    # PE slack (only rows already normalized)
                    if c_ready[0] > 0 and c_queue and i >= 2:
                        c_ready[0] -= 1
                        emit_c_group()

                # ---- normalize ----
                rs2 = rspool.tile([65, 512], FR, tag="rs", name=f"rs_{j}_{g}")
                for h in range(2):
                    nc.vector.tensor_copy(rs2[64 * h:64 * h + 1, :],
                                          po[64:65, h, :])
                rbb = rbpool.tile([64, 2, 512], F32, tag="rb",
                                  name=f"rbb_{j}_{g}")
                pru = pspool.tile([P, 2, 512], F32, tag="ps",
                                  name=f"pru_{j}_{g}")
                for h in range(2):
                    nc.tensor.matmul(pru[0:64, h, :],
                                     ones_sb[64 * h:64 * h + 1, :],
                                     rs2[64 * h:64 * h + 1, :],
                                     start=True, stop=True)
                nc.vector.reciprocal_approx_fast(rbb[:], pru[0:64, :, :])
                for h in range(2):
                    nc.vector.tensor_mul(
                        oT_sb[h * 64:(h + 1) * 64, g, qs0:qs0 + 512],
                        po[0:64, h, :], rbb[:, h, :])
            # rows 4j..4j+3 of oT are fully normalized after all g done:
            # release 16 output-projection groups
            c_ready[0] += 4 * NT

        while c_queue:
            emit_c_group()


def build_nc():
    global _NC_CACHE
    if _NC_CACHE is not None:
        return _NC_CACHE
    nc = bacc.Bacc("TRN2", debug=False, target_bir_lowering=False,
                   num_devices=NCORES)
    ins = {
        "xT": nc.dram_tensor("xT", [C, T], BF16, kind="ExternalInput").ap(),
        "wqT": nc.dram_tensor("wqT", [C, 512], BF16, kind="ExternalInput").ap(),
        "wkT": nc.dram_tensor("wkT", [C, P], BF16, kind="ExternalInput").ap(),
        "wvT": nc.dram_tensor("wvT", [C, P], BF16, kind="ExternalInput").ap(),
        "woR": nc.dram_tensor("woR", [512, C], BF16, kind="ExternalInput").ap(),
        "tri": nc.dram_tensor("tri", [P, P], BF16, kind="ExternalInput").ap(),
        "iden": nc.dram_tensor("iden", [P, P], BF16, kind="ExternalInput").ap(),
        "ones64": nc.dram_tensor("ones64", [65, 64], FR,
                                 kind="ExternalInput").ap(),
        "vinit": nc.dram_tensor("vinit", [P, NTT, 130], BF16,
                                kind="ExternalInput").ap(),
    }
    outs = {"y": nc.dram_tensor("y", [T, C], BF16, kind="ExternalOutput").ap()}
    with tile.TileContext(nc) as tc:
        with tc.tile_pool(name="persist", bufs=1) as persist:
            tc._persist_pool = persist
            build_kernel(nc, tc, ins, outs)
    nc.compile()
    _NC_CACHE = nc
    return nc


def make_core_inputs(x, Wq, Wkv, Wo):
    """Host-side shard + pre-transpose + bf16 cast. Returns list of 8 in_maps."""
    bf = ml_dtypes.bfloat16
    x = np.asarray(x, np.float32)
    Wq = np.asarray(Wq, np.float32)
    Wkv = np.asarray(Wkv, np.float32)
    Wo = np.asarray(Wo, np.float32)
    tri = np.where(np.tri(P, P, -1, dtype=bool), -1.0e5,
                   0.0).astype(bf)  # -1e5 where key > query
    iden = np.eye(P, dtype=np.float32).astype(bf)
    vinit = np.zeros((P, NTT, 130), np.float32)
    vinit[:, :, 64] = 1.0
    vinit[:, :, 129] = 1.0
    in_maps = []
    for c in range(NCORES):
        b, gc = c // 4, c % 4
        xT = np.ascontiguousarray(x[b].T).astype(bf)             # [C, T]
        Wq4 = Wq.reshape(HKV, G, D, C)[2 * gc:2 * gc + 2]        # [2, G, D, C]
        wqT = np.ascontiguousarray(
            np.transpose(Wq4, (1, 0, 2, 3)).reshape(512, C).T).astype(bf)
        wkT = np.ascontiguousarray(
            Wkv[2 * gc * 64:(2 * gc + 2) * 64].T).astype(bf)
        wvT = np.ascontiguousarray(
            Wkv[HKV * D + 2 * gc * 64:HKV * D + (2 * gc + 2) * 64].T).astype(bf)
        Wo4 = Wo.reshape(C, HKV, G, D)[:, 2 * gc:2 * gc + 2]     # [C, 2, G, D]
        woR = np.ascontiguousarray(
            np.transpose(Wo4, (2, 1, 3, 0)).reshape(512, C)).astype(bf)
        in_maps.append({"xT": xT, "wqT": wqT, "wkT": wkT, "wvT": wvT,
                        "woR": woR, "tri": tri, "iden": iden,
                        "ones64": np.ones((65, 64), np.float32),
                        "vinit": vinit.astype(bf)})
    return in_maps


def kernel(x, Wq, Wkv, Wo, trace=False):
    nc = build_nc()
    in_maps = make_core_inputs(x, Wq, Wkv, Wo)
    res = run_bass_kernel_spmd(nc, in_maps, core_ids=list(range(NCORES)),
                               trace=trace)
    y = np.zeros((B, T, C), np.float32)
    for c in range(NCORES):
        y[c // 4] += np.asarray(res.results[c]["y"], np.float32)
    if trace:
        kernel.last_exec_time_ns = res.exec_time_ns
        kernel.last_results = res
    return y


# revision 12
# speedup vs baseline: 1.3837x; 1.3837x over previous
"""GroupedQueryAttention TRN2 kernel (v4).

Sharding: 4-way tensor-parallel over heads x 2-way data-parallel over batch.
Core c handles batch b=c//4 and head-group gc=c%4 (kv heads {2gc, 2gc+1},
q heads (hkv, g) for g in 0..3 -> 8 q heads per core).

HW reality (microbenched): PE streams ~0.51ns/col for ANY dtype with a
~196ns pipeline-restart cost at dependency waits; ScalarE exp of a
[128,2,512] tile costs ~1.2us and paces phase B. The Tile scheduler
reorders per-engine instructions, so throughput is governed by ring
depths (buffering), not emission order.

Per-core program (T=2048, C=2048, D=64), matmuls bf16 (fp32 PSUM except S):
  A: Q^T/K^T/V^T projections; weight tiles stream from the GpSimd DGE
     queue so the x-tile stream on the Sync queue starts immediately.
  B: S tiles in FP16 PSUM (1 bank each -> ps ring depth 3, deep exp
     lookahead). Causal mask: GpSimd zeroes the exp'd diagonal corners
     (0/1 triangle multiply, latency hidden by the ring). O'^T + rowsum
     accumulated on PE via the V' ones-column. Normalize: DVE rowsum
     copy, PE broadcast into a dedicated 1-bank pru, DVE
     reciprocal_approx_fast, DVE multiply -> oT bf16.
  C: y-groups (4 matmuls into a borrowed ps-ring slot, DVE evac, DMA out)
     interleaved into B to fill ScalarE-bound slack; leftovers drain after.
  Host sums 4 bf16 partials per batch.
"""

import numpy as np
import ml_dtypes

import concourse.bass as bass
import concourse.mybir as mybir
import concourse.tile as tile
from concourse import bacc
from concourse.bass_utils import run_bass_kernel_spmd

H, HKV, D, G = 32, 8, 64, 4
B, T, C = 2, 2048, 2048
P = 128
NCORES = 8
F32 = mybir.dt.float32
FR = mybir.dt.float32r
BF16 = mybir.dt.bfloat16
FP16 = mybir.dt.float16

NT = T // 512   # 4 q blocks of 512
NK = C // P     # 16 contraction tiles
NTT = T // P    # 16 key/row tiles

_NC_CACHE = None


def build_kernel(nc, tc, ins, outs):
    xT, wqT, wkT, wvT, woR = (
        ins["xT"], ins["wqT"], ins["wkT"], ins["wvT"], ins["woR"])
    tri, iden = ins["tri"], ins["iden"]
    y = outs["y"]
    EXP = mybir.ActivationFunctionType.Exp
    CPY = mybir.ActivationFunctionType.Copy

    # ---- persistent SBUF ----
    persist = tc._persist_pool
    qT_sb = persist.tile([P, G, T], BF16, name="qT_sb", tag="qT_sb")
    kT_sb = persist.tile([P, T], BF16, name="kT_sb", tag="kT_sb")
    vTt = persist.tile([P, T], BF16, name="vTt", tag="vTt")
    v_sb = persist.tile([P, NTT, 130], BF16, name="v_sb", tag="v_sb")
    oT_sb = persist.tile([P, G, T], BF16, name="oT_sb", tag="oT_sb")
    wo_sb = persist.tile([P, G, C], BF16, name="wo_sb", tag="wo_sb")
    tri_sb = persist.tile([P, P], BF16, name="tri_sb", tag="tri_sb")
    iden_sb = persist.tile([P, P], BF16, name="iden_sb", tag="iden_sb")
    ones_sb = persist.tile([65, 64], FR, name="ones_sb", tag="ones_sb")

    nc.sync.dma_start(iden_sb[:], iden[:])
    nc.sync.dma_start(tri_sb[:], tri[:])
    nc.sync.dma_start(ones_sb[:], ins["ones64"][:])
    nc.sync.dma_start(v_sb[:], ins["vinit"][:])

    # ================= phase A: projections =================
    with (
        tc.tile_pool(name="wproj", bufs=1) as wpool,
        tc.tile_pool(name="xt", bufs=6) as xpool,
        tc.tile_pool(name="pp", bufs=8, space="PSUM") as pp,
    ):
        wq_sb = wpool.tile([P, NK, 512], BF16, name="wq_sb")
        wk_sb = wpool.tile([P, NK, P], BF16, name="wk_sb")
        wv_sb = wpool.tile([P, NK, P], BF16, name="wv_sb")
        wqr = wqT.rearrange("(ko p) m -> p ko m", p=P)
        wkr = wkT.rearrange("(ko p) m -> p ko m", p=P)
        wvr = wvT.rearrange("(ko p) m -> p ko m", p=P)
        # weight slices stream from the GpSimd DGE queue; x tiles own Sync
        for k in range(NK):
            nc.gpsimd.dma_start(wq_sb[:, k, :], wqr[:, k, :])
            nc.gpsimd.dma_start(wk_sb[:, k, :], wkr[:, k, :])
            nc.gpsimd.dma_start(wv_sb[:, k, :], wvr[:, k, :])

        for tb in range(NT):
            ts = slice(tb * 512, (tb + 1) * 512)
            psq = [pp.tile([P, 512], F32, tag="pp", name=f"psq_{tb}_{g}")
                   for g in range(G)]
            psk = pp.tile([P, 512], F32, tag="pp", name=f"psk_{tb}")
            psv = pp.tile([P, 512], F32, tag="pp", name=f"psv_{tb}")
            for k in range(NK):
                xt = xpool.tile([P, 512], BF16, tag="xt", name=f"xt_{tb}_{k}")
                nc.sync.dma_start(xt[:], xT[k * P:(k + 1) * P, ts])
                st, sp = (k == 0), (k == NK - 1)
                for g in range(G):
                    nc.tensor.matmul(psq[g][:],
                                     wq_sb[:, k, g * P:(g + 1) * P],
                                     xt[:], start=st, stop=sp)
                nc.tensor.matmul(psk[:], wk_sb[:, k, :], xt[:],
                                 start=st, stop=sp)
                nc.tensor.matmul(psv[:], wv_sb[:, k, :], xt[:],
                                 start=st, stop=sp)
            for g in range(G):
                nc.scalar.activation(qT_sb[:, g, ts], psq[g][:], CPY)
            nc.vector.tensor_copy(kT_sb[:, ts], psk[:])
            nc.vector.tensor_copy(vTt[:, ts], psv[:])
            for tt in range(4 * tb, 4 * tb + 4):
                pt_ = pp.tile([P, P], BF16, tag="pp", name=f"pvt_{tt}")
                nc.tensor.transpose(pt_[:], vTt[:, tt * P:(tt + 1) * P],
                                    iden_sb[:])
                nc.vector.tensor_copy(v_sb[:, tt, 0:64], pt_[:, 0:64])
                nc.vector.tensor_copy(v_sb[:, tt, 65:129], pt_[:, 64:128])

    # ================= phase B: attention (two streams) =================
    nc.gpsimd.dma_start(wo_sb[:], woR.rearrange("(m p) c -> p m c", p=P))
    with (
        tc.tile_pool(name="psA", bufs=1, space="PSUM") as psA,
        tc.tile_pool(name="psB", bufs=1, space="PSUM") as psB,
        tc.tile_pool(name="po", bufs=2, space="PSUM") as popool,
        tc.tile_pool(name="ptp", bufs=8) as ptpool,
        tc.tile_pool(name="rs", bufs=4) as rspool,
        tc.tile_pool(name="rb", bufs=4) as rbpool,
    ):
        for j in range(NT):
            qs0 = j * 512
            n = 4 * j + 4
            for gp in range(2):
                streams = [(2 * gp, psA), (2 * gp + 1, psB)]
                po = {}
                for g, _pool in streams:
                    po[g] = popool.tile([P, 2, 512], F32, tag="po",
                                        name=f"po_{j}_{g}")
                for i in range(n):
                    loc = max(0, P * i - qs0)
                    diag = i >= 4 * j
                    for g, pool in streams:
                        ps = pool.tile([P, 2, 512], F32, tag="s",
                                       name=f"ps_{j}_{g}_{i}")
                        for h in range(2):
                            nc.tensor.matmul(
                                ps[:, h, loc:512],
                                kT_sb[h * 64:(h + 1) * 64,
                                      i * P:(i + 1) * P],
                                qT_sb[h * 64:(h + 1) * 64, g,
                                      qs0 + loc:qs0 + 512],
                                start=True, stop=True,
                            )
                        pt_ = ptpool.tile([P, 2, 512], BF16, tag="pt",
                                          name=f"pt_{j}_{g}_{i}")
                        nc.scalar.activation(pt_[:, :, loc:512],
                                             ps[:, :, loc:512],
                                             EXP, scale=0.125)
                        if diag:  # zero the masked corner (GpSimd)
                            nc.gpsimd.tensor_mul(
                                pt_[:, :, loc:loc + P],
                                pt_[:, :, loc:loc + P],
                                tri_sb[:, None, :].to_broadcast([P, 2, P]),
                            )
                        for h in range(2):
                            nc.tensor.matmul(
                                po[g][0:65, h, loc:512],
                                v_sb[:, i, h * 65:h * 65 + 65],
                                pt_[:, h, loc:512],
                                start=(i == 0), stop=(i == n - 1),
                            )
                # ---- normalize both streams ----
                for g, pool in streams:
                    rs2 = rspool.tile([65, 512], FR, tag="rs",
                                      name=f"rs_{j}_{g}")
                    for h in range(2):
                        nc.vector.tensor_copy(rs2[64 * h:64 * h + 1, :],
                                              po[g][64:65, h, :])
                    pru = pool.tile([P, 2, 512], F32, tag="s",
                                    name=f"pru_{j}_{g}")
                    for h in range(2):
                        nc.tensor.matmul(pru[0:64, h, :],
                                         ones_sb[64 * h:64 * h + 1, :],
                                         rs2[64 * h:64 * h + 1, :],
                                         start=True, stop=True)
                    rbb = rbpool.tile([64, 2, 512], F32, tag="rb",
                                      name=f"rbb_{j}_{g}")
                    nc.vector.reciprocal_approx_fast(rbb[:],
                                                     pru[0:64, :, :])
                    for h in range(2):
                        nc.vector.tensor_mul(
                            oT_sb[h * 64:(h + 1) * 64, g, qs0:qs0 + 512],
                            po[g][0:64, h, :], rbb[:, h, :])

    # ================= phase C: output projection =================
    with (
        tc.tile_pool(name="py", bufs=6, space="PSUM") as pypool,
        tc.tile_pool(name="ysb", bufs=6) as ypool,
    ):
        for tt in range(NTT):
            for cb in range(NT):
                py = pypool.tile([P, 512], F32, tag="py", name=f"py_{tt}_{cb}")
                for m in range(G):
                    nc.tensor.matmul(
                        py[:], oT_sb[:, m, tt * P:(tt + 1) * P],
                        wo_sb[:, m, cb * 512:(cb + 1) * 512],
                        start=(m == 0), stop=(m == G - 1),
                    )
                ysb = ypool.tile([P, 512], BF16, tag="ysb", name=f"y_{tt}_{cb}")
                if (tt * NT + cb) % 2 == 0:
                    nc.scalar.activation(ysb[:], py[:], CPY)
                else:
                    nc.vector.tensor_copy(ysb[:], py[:])
                nc.sync.dma_start(y[tt * P:(tt + 1) * P, cb * 512:(cb + 1) * 512],
                                  ysb[:])


def build_nc():
    global _NC_CACHE
    if _NC_CACHE is not None:
        return _NC_CACHE
    nc = bacc.Bacc("TRN2", debug=False, target_bir_lowering=False,
                   num_devices=NCORES)
    ins = {
        "xT": nc.dram_tensor("xT", [C, T], BF16, kind="ExternalInput").ap(),
        "wqT": nc.dram_tensor("wqT", [C, 512], BF16, kind="ExternalInput").ap(),
        "wkT": nc.dram_tensor("wkT", [C, P], BF16, kind="ExternalInput").ap(),
        "wvT": nc.dram_tensor("wvT", [C, P], BF16, kind="ExternalInput").ap(),
        "woR": nc.dram_tensor("woR", [512, C], BF16, kind="ExternalInput").ap(),
        "tri": nc.dram_tensor("tri", [P, P], BF16, kind="ExternalInput").ap(),
        "iden": nc.dram_tensor("iden", [P, P], BF16, kind="ExternalInput").ap(),
        "ones64": nc.dram_tensor("ones64", [65, 64], FR,
                                 kind="ExternalInput").ap(),
        "vinit": nc.dram_tensor("vinit", [P, NTT, 130], BF16,
                                kind="ExternalInput").ap(),
    }
    outs = {"y": nc.dram_tensor("y", [T, C], BF16, kind="ExternalOutput").ap()}
    with tile.TileContext(nc) as tc:
        with tc.tile_pool(name="persist", bufs=1) as persist:
            tc._persist_pool = persist
            build_kernel(nc, tc, ins, outs)
    nc.compile()
    _NC_CACHE = nc
    return nc


def make_core_inputs(x, Wq, Wkv, Wo):
    """Host-side shard + pre-transpose + bf16 cast. Returns list of 8 in_maps."""
    bf = ml_dtypes.bfloat16
    x = np.asarray(x, np.float32)
    Wq = np.asarray(Wq, np.float32)
    Wkv = np.asarray(Wkv, np.float32)
    Wo = np.asarray(Wo, np.float32)
    tri = np.triu(np.ones((P, P), np.float32)).astype(bf)  # keep q >= k
    iden = np.eye(P, dtype=np.float32).astype(bf)
    vinit = np.zeros((P, NTT, 130), np.float32)
    vinit[:, :, 64] = 1.0
    vinit[:, :, 129] = 1.0
    in_maps = []
    for c in range(NCORES):
        b, gc = c // 4, c % 4
        xT = np.ascontiguousarray(x[b].T).astype(bf)             # [C, T]
        Wq4 = Wq.reshape(HKV, G, D, C)[2 * gc:2 * gc + 2]        # [2, G, D, C]
        wqT = np.ascontiguousarray(
            np.transpose(Wq4, (1, 0, 2, 3)).reshape(512, C).T).astype(bf)
        wkT = np.ascontiguousarray(
            Wkv[2 * gc * 64:(2 * gc + 2) * 64].T).astype(bf)
        wvT = np.ascontiguousarray(
            Wkv[HKV * D + 2 * gc * 64:HKV * D + (2 * gc + 2) * 64].T).astype(bf)
        Wo4 = Wo.reshape(C, HKV, G, D)[:, 2 * gc:2 * gc + 2]     # [C, 2, G, D]
        woR = np.ascontiguousarray(
            np.transpose(Wo4, (2, 1, 3, 0)).reshape(512, C)).astype(bf)
        in_maps.append({"xT": xT, "wqT": wqT, "wkT": wkT, "wvT": wvT,
                        "woR": woR, "tri": tri, "iden": iden,
                        "ones64": np.ones((65, 64), np.float32),
                        "vinit": vinit.astype(bf)})
    return in_maps


def kernel(x, Wq, Wkv, Wo, trace=False):
    nc = build_nc()
    in_maps = make_core_inputs(x, Wq, Wkv, Wo)
    res = run_bass_kernel_spmd(nc, in_maps, core_ids=list(range(NCORES)),
                               trace=trace)
    y = np.zeros((B, T, C), np.float32)
    for c in range(NCORES):
        y[c // 4] += np.asarray(res.results[c]["y"], np.float32)
    if trace:
        kernel.last_exec_time_ns = res.exec_time_ns
        kernel.last_results = res
    return y
